# revision 3
# baseline (speedup 1.0000x reference)
"""Local windowed MHA (lucidrains LocalAttention, window=128, look_back=1,
look_fwd=1, non-causal) on 8 TRN2 NeuronCores.

Sharding: core = batch*2 + seq_half. Each core owns 4096 tokens of one
batch element plus a 128-token halo on each side (zero-padded at true
sequence edges). Attention is local, so shards are fully independent —
no collectives.

Per-core dataflow (zero on-chip transposes):
  host passes xT (512, 4352) bf16, w_qkvT (512, 1536) bf16,
  w_outT (512, 512) bf16, cw (1, 32) f32.
  - q_fm, k_fm: feature-major [feat, tok] projections (lhsT = w slice).
  - v_tm: token-major [tok, feat] projection, stored per window as
    [128tok, 8head, 65] with column 64 = 1.0 (ones column).
  - sim_T[j, i] = k_fm.T @ q_fm per (window, head, key-window).
  - e = exp(sim_T / 8) (no max subtraction: scores are O(1); pad keys
    give sim=0 -> e=1).
  - A@V: lhsT = v65 [j, 65], rhs = e [j, i] -> psum [65, i]: rows 0..63
    = unnormalized head output (feature-major), row 64 = sum_j e = raw
    softmax denominator.
  - denominator correction: pad keys contribute exactly 1.0 each; host
    passes cw[w] in {0, 128}; den = raw - cw. Pad v rows are exactly 0
    so the numerator needs no correction. This reproduces the reference
    mask exactly.
  - normalize: recip(den) broadcast across partitions via a tiny K=2
    indicator matmul on the PE; DVE multiply -> att feature-major bf16.
  - out-proj: lhsT = att chunk [128e, 128i], rhs = w_outT chunk
    [128e, 512m] -> psum [128i, 512m] token-major; DMA contiguous.
"""

import sys

sys.path.insert(0, "/opt/trn_rl_repo")

import numpy as np
import ml_dtypes

import concourse.bass as bass
import concourse.tile as tile
import concourse.mybir as mybir
from concourse import bacc
from concourse.bass_utils import run_bass_kernel_spmd

P = 128
HEADS = 8
DH = 64
W = 128  # window size
D = 512  # model dim
B = 4
SEQ = 8192
OWN = 4096  # tokens owned per core
HALO = 128
EXT = OWN + 2 * HALO  # 4352
NWIN = EXT // W  # 34 windows in shard (0 and 33 are halo)
OWIN = OWN // W  # 32 owned windows
BF16 = mybir.dt.bfloat16
F32 = mybir.dt.float32
SCALE = DH ** -0.5  # 0.125

_cached = {}


def _build_program():
    nc = bacc.Bacc("TRN2", target_bir_lowering=False, debug=False, num_devices=8)

    xT = nc.dram_tensor("xt", [D, EXT], BF16, kind="ExternalInput").ap()
    wqkvT = nc.dram_tensor("wqkvt", [D, 3 * D], BF16, kind="ExternalInput").ap()
    woutT = nc.dram_tensor("woutt", [D, D], BF16, kind="ExternalInput").ap()
    cw = nc.dram_tensor("cw", [1, OWIN], F32, kind="ExternalInput").ap()
    out = nc.dram_tensor("out", [OWN, D], F32, kind="ExternalOutput").ap()

    with tile.TileContext(nc) as tc:
        _emit(tc, xT, wqkvT, woutT, cw, out)

    nc.compile()
    return nc


def _emit(tc, xT, wqkvT, woutT, cw, out):
    nc = tc.nc
    import contextlib

    ctx = contextlib.ExitStack()
    with ctx:
        const = ctx.enter_context(tc.tile_pool(name="const", bufs=1))
        proj_ps = ctx.enter_context(tc.tile_pool(name="proj_ps", bufs=2, space="PSUM"))
        sim_ps = ctx.enter_context(tc.tile_pool(name="sim_ps", bufs=2, space="PSUM"))
        att_ps = ctx.enter_context(tc.tile_pool(name="att_ps", bufs=2, space="PSUM"))
        bc_ps = ctx.enter_context(tc.tile_pool(name="bc_ps", bufs=1, space="PSUM"))
        out_ps = ctx.enter_context(tc.tile_pool(name="out_ps", bufs=1, space="PSUM"))
        epool = ctx.enter_context(tc.tile_pool(name="epool", bufs=3))
        spool = ctx.enter_context(tc.tile_pool(name="spool", bufs=3))
        opool = ctx.enter_context(tc.tile_pool(name="opool", bufs=2))

        # ---- persistent SBUF tensors ----
        x_sb = const.tile([P, 4, EXT], BF16)  # x feature-major: feat s*128+p
        w_sb = const.tile([P, 4, 3 * D], BF16)  # w_qkvT: [d_in, e_out]
        wo_sb = const.tile([P, 4, D], BF16)  # w_outT: [e_in, m_out]
        k_sb = const.tile([P, 4, EXT], BF16)  # k feature-major
        q_sb = const.tile([P, 4, OWN], BF16)  # q feature-major (owned only)
        v_sb = const.tile([P, NWIN, HEADS, DH + 1], BF16)  # v token-major + ones col
        cw_sb = const.tile([1, OWIN], F32)
        ones1 = const.tile([1, DH], F32)  # K=1 lhsT for denom broadcast

        nc.sync.dma_start(x_sb[:], xT.rearrange("(s p) t -> p s t", p=P))
        nc.sync.dma_start(w_sb[:], wqkvT.rearrange("(s p) e -> p s e", p=P))
        nc.sync.dma_start(wo_sb[:], woutT.rearrange("(s p) m -> p s m", p=P))
        nc.sync.dma_start(cw_sb[:], cw[:])

        nc.vector.memset(ones1[:], 1.0)

        # ---- projections ----
        # k feature-major: lhsT = w_k chunk [128d, 128e], rhs = x [128d, Tt]
        TB = 512
        nblk = (EXT + TB - 1) // TB  # 9 (last block 256)
        for blk in range(nblk):
            t0 = blk * TB
            tb = min(TB, EXT - t0)
            for ec in range(4):
                ps = proj_ps.tile([P, TB], F32, tag="proj")
                for s in range(4):
                    nc.tensor.matmul(
                        ps[:, :tb],
                        lhsT=w_sb[:, s, D + ec * P : D + (ec + 1) * P],
                        rhs=x_sb[:, s, t0 : t0 + tb],
                        start=(s == 0),
                        stop=(s == 3),
                    )
                nc.vector.tensor_copy(k_sb[:, ec, t0 : t0 + tb], ps[:, :tb])

        # q feature-major, owned tokens only (shard tokens 128..4224)
        for blk in range(OWN // TB):  # 8
            t0 = blk * TB
            for ec in range(4):
                ps = proj_ps.tile([P, TB], F32, tag="proj")
                for s in range(4):
                    nc.tensor.matmul(
                        ps[:],
                        lhsT=w_sb[:, s, ec * P : (ec + 1) * P],
                        rhs=x_sb[:, s, HALO + t0 : HALO + t0 + TB],
                        start=(s == 0),
                        stop=(s == 3),
                    )
                nc.vector.tensor_copy(q_sb[:, ec, t0 : t0 + TB], ps[:])

        # v token-major per window: lhsT = x tile [128d, 128t], rhs = w_v [128d, 512e]
        for w in range(NWIN):
            ps = proj_ps.tile([P, TB], F32, tag="proj")
            for s in range(4):
                nc.tensor.matmul(
                    ps[:],
                    lhsT=x_sb[:, s, w * W : (w + 1) * W],
                    rhs=w_sb[:, s, 2 * D : 3 * D],
                    start=(s == 0),
                    stop=(s == 3),
                )
            # strided copy into [128, 8, 65] slots (cols 0..63 per head)
            nc.vector.tensor_copy(
                v_sb[:, w, :, 0:DH], ps.rearrange("p (h e) -> p h e", h=HEADS)
            )
            nc.vector.memset(v_sb[:, w, :, DH : DH + 1], 1.0)

        # ---- attention ----
        for w in range(1, NWIN - 1):  # owned windows
            wi = w - 1
            out_psum = out_ps.tile([P, D], F32, tag="outp")
            att_sb = spool.tile([P, 4, W], BF16, tag="att")
            for c in range(4):  # head pairs (2c, 2c+1)
                att_psums = []
                bc = bc_ps.tile([P, W], F32, tag="bc")
                for hh in range(2):
                    h = 2 * c + hh
                    off = hh * DH
                    sim = sim_ps.tile([P, 3, W], F32, tag="sim")
                    for kwi in range(3):
                        kw = w - 1 + kwi
                        nc.tensor.matmul(
                            sim[:, kwi, :],
                            lhsT=k_sb[off : off + DH, c, kw * W : (kw + 1) * W],
                            rhs=q_sb[off : off + DH, c, wi * W : (wi + 1) * W],
                            start=True,
                            stop=True,
                        )
                    e = epool.tile([P, 3, W], BF16, tag="e")
                    nc.scalar.activation(
                        e[:], sim[:], mybir.ActivationFunctionType.Exp, scale=SCALE
                    )
                    ap = att_ps.tile([DH + 1, W], F32, tag="attp")
                    for kwi in range(3):
                        kw = w - 1 + kwi
                        nc.tensor.matmul(
                            ap[:],
                            lhsT=v_sb[:, kw, h, :],
                            rhs=e[:, kwi, :],
                            start=(kwi == 0),
                            stop=(kwi == 2),
                        )
                    att_psums.append(ap)
                    # den = raw_denominator - cw[wi]
                    den = spool.tile([1, W], F32, tag="den")
                    nc.vector.tensor_scalar(
                        den[:],
                        ap[DH : DH + 1, :],
                        cw_sb[0:1, wi : wi + 1],
                        None,
                        mybir.AluOpType.subtract,
                    )
                    recip = spool.tile([1, W], F32, tag="recip")
                    nc.vector.reciprocal(recip[:], den[:])
                    # replicate recip across 64 partitions via K=1 matmul
                    nc.tensor.matmul(
                        bc[off : off + DH, :],
                        lhsT=ones1[:],
                        rhs=recip[:],
                        start=True,
                        stop=True,
                    )
                bc_sb = spool.tile([P, W], F32, tag="bcsb")
                nc.vector.tensor_copy(bc_sb[:], bc[:])
                for hh in range(2):
                    nc.vector.tensor_tensor(
                        att_sb[hh * DH : (hh + 1) * DH, c, :],
                        att_psums[hh][0:DH, :],
                        bc_sb[hh * DH : (hh + 1) * DH, :],
                        mybir.AluOpType.mult,
                    )
                nc.tensor.matmul(
                    out_psum[:],
                    lhsT=att_sb[:, c, :],
                    rhs=wo_sb[:, c, :],
                    start=(c == 0),
                    stop=(c == 3),
                )
            out_sb = opool.tile([P, D], F32, tag="osb")
            nc.vector.tensor_copy(out_sb[:], out_psum[:])
            nc.sync.dma_start(out[wi * W : (wi + 1) * W, :], out_sb[:])


def _get_program():
    if "nc" not in _cached:
        _cached["nc"] = _build_program()
    return _cached["nc"]


def _make_in_maps(x, w_qkv, w_out):
    bf16 = ml_dtypes.bfloat16
    wqkvT = np.ascontiguousarray(np.asarray(w_qkv, np.float32).T).astype(bf16)
    woutT = np.ascontiguousarray(np.asarray(w_out, np.float32).T).astype(bf16)
    x = np.asarray(x, np.float32)
    in_maps = []
    for core in range(8):
        b, half = core // 2, core % 2
        s = half * OWN
        lo, hi = s - HALO, s + OWN + HALO
        xs = np.zeros((EXT, D), np.float32)
        src_lo, src_hi = max(lo, 0), min(hi, SEQ)
        xs[src_lo - lo : src_hi - lo] = x[b, src_lo:src_hi]
        xTc = np.ascontiguousarray(xs.T).astype(bf16)
        cwv = np.zeros((1, OWIN), np.float32)
        if half == 0:
            cwv[0, 0] = float(HALO)
        else:
            cwv[0, OWIN - 1] = float(HALO)
        in_maps.append({"xt": xTc, "wqkvt": wqkvT, "woutt": woutT, "cw": cwv})
    return in_maps


def run(x, w_qkv, w_out, trace=False, **spmd_kwargs):
    nc = _get_program()
    in_maps = _make_in_maps(x, w_qkv, w_out)
    res = run_bass_kernel_spmd(
        nc, in_maps, list(range(8)), trace=trace, **spmd_kwargs
    )
    out = np.empty((B, SEQ, D), np.float32)
    for core in range(8):
        b, half = core // 2, core % 2
        out[b, half * OWN : (half + 1) * OWN] = res.results[core]["out"]
    return out, res


def kernel(x, w_qkv, w_out):
    out, _ = run(x, w_qkv, w_out)
    return out


# revision 4
# speedup vs baseline: 2.2383x; 2.2383x over previous
"""Local windowed MHA (lucidrains LocalAttention, window=128, look_back=1,
look_fwd=1, non-causal) on 8 TRN2 NeuronCores.

Sharding: core = batch*2 + seq_half. Each core owns 4096 tokens of one
batch element plus a 128-token halo on each side (zero-padded at true
sequence edges). Attention is local, so shards are fully independent —
no collectives.

Per-core dataflow (zero DMA transposes; one PE transpose per head-pair):
  host passes xT (512, 4352) bf16, w_qkvT (512, 1536) bf16,
  w_outT (512, 512) bf16, vones (4352,) bf16 (1.0 for real tokens,
  0.0 for out-of-sequence pad tokens).
  - q_fm, k_fm: feature-major [feat, tok] projections (lhsT = w slice).
  - v65: token-major [128tok, head, 65] with column 64 = vones (pad
    indicator). Pad x is zero, so pad k and v are exactly zero.
  - QK, kw-batched: per (key-window kw, head): ONE matmul
    lhsT=k[kw] [64, 128], rhs=q over all query windows attending kw
    (N<=384) -> sim_T [128j, i] in psum; exp via ACT (scale=1/8, no max
    subtraction needed: |sim/8| < ~2).
  - A@V token-major: lhsT = e slice [128j, 128i], rhs = v65 [128j, 65]
    -> psum att[128i, head, 65] accumulated over 3 kws. Column 64 =
    sum_j e[j,i]*vones[j] = exact softmax denominator (pad keys excluded
    because their indicator is 0; pad v rows are exactly 0 so the
    numerator needs no correction). Reproduces the reference mask
    exactly.
  - normalize: one reciprocal [128, 8] + one multiply [128, 8, 64]
    per window -> attn token-major bf16.
  - PE-transpose per head-pair [128, 128] -> feature-major chunks;
    out-proj lhsT = chunk [128e, 128i], rhs = w_outT chunk [128e, 512m]
    -> psum [128i, 512m] token-major; contiguous DMA out.
"""

import sys

sys.path.insert(0, "/opt/trn_rl_repo")

import numpy as np
import ml_dtypes

import concourse.bass as bass
import concourse.tile as tile
import concourse.mybir as mybir
from concourse import bacc
from concourse.bass_utils import run_bass_kernel_spmd
from concourse.masks import make_identity

P = 128
HEADS = 8
DH = 64
W = 128  # window size
D = 512  # model dim
B = 4
SEQ = 8192
OWN = 4096  # tokens owned per core
HALO = 128
EXT = OWN + 2 * HALO  # 4352
NWIN = EXT // W  # 34 windows in shard (0 and 33 are halo)
OWIN = OWN // W  # 32 owned windows
BF16 = mybir.dt.bfloat16
F32 = mybir.dt.float32
SCALE = DH ** -0.5  # 0.125

_cached = {}


def _build_program():
    nc = bacc.Bacc("TRN2", target_bir_lowering=False, debug=False, num_devices=8)

    xT = nc.dram_tensor("xt", [D, EXT], BF16, kind="ExternalInput").ap()
    wqkvT = nc.dram_tensor("wqkvt", [D, 3 * D], BF16, kind="ExternalInput").ap()
    woutT = nc.dram_tensor("woutt", [D, D], BF16, kind="ExternalInput").ap()
    vones = nc.dram_tensor("vones", [EXT], BF16, kind="ExternalInput").ap()
    out = nc.dram_tensor("out", [OWN, D], F32, kind="ExternalOutput").ap()

    with tile.TileContext(nc) as tc:
        _emit(tc, xT, wqkvT, woutT, vones, out)

    nc.compile()
    return nc


def _emit(tc, xT, wqkvT, woutT, vones, out):
    nc = tc.nc
    import contextlib

    ctx = contextlib.ExitStack()
    with ctx:
        const = ctx.enter_context(tc.tile_pool(name="const", bufs=1))
        # PSUM budget (8 banks): sim pool 2 bufs x 2 banks = 4 (proj shares
        # the "sim" tag), att 1 x 2, tr 1 x <1, out 1 x 1.
        sim_ps = ctx.enter_context(tc.tile_pool(name="sim_ps", bufs=2, space="PSUM"))
        att_ps = ctx.enter_context(tc.tile_pool(name="att_ps", bufs=1, space="PSUM"))
        tr_ps = ctx.enter_context(tc.tile_pool(name="tr_ps", bufs=1, space="PSUM"))
        out_ps = ctx.enter_context(tc.tile_pool(name="out_ps", bufs=1, space="PSUM"))
        epool = ctx.enter_context(tc.tile_pool(name="epool", bufs=14))
        spool = ctx.enter_context(tc.tile_pool(name="spool", bufs=3))
        opool = ctx.enter_context(tc.tile_pool(name="opool", bufs=2))

        # ---- persistent SBUF tensors ----
        x_sb = const.tile([P, 4, EXT], BF16)  # x feature-major: feat s*128+p
        w_sb = const.tile([P, 4, 3 * D], BF16)  # w_qkvT: [d_in, e_out]
        wo_sb = const.tile([P, 4, D], BF16)  # w_outT: [e_in, m_out]
        k_sb = const.tile([P, 4, EXT], BF16)  # k feature-major
        q_sb = const.tile([P, 4, OWN], BF16)  # q feature-major (owned only)
        v_sb = const.tile([P, NWIN, HEADS, DH + 1], BF16)  # v tok-major + den col
        vo_sb = const.tile([P, NWIN], BF16)  # pad indicator per (tok%128, win)
        ident = const.tile([P, P], BF16)

        nc.sync.dma_start(x_sb[:], xT.rearrange("(s p) t -> p s t", p=P))
        nc.sync.dma_start(w_sb[:], wqkvT.rearrange("(s p) e -> p s e", p=P))
        nc.sync.dma_start(wo_sb[:], woutT.rearrange("(s p) m -> p s m", p=P))
        nc.sync.dma_start(vo_sb[:], vones.rearrange("(w p) -> p w", p=P))
        make_identity(nc, ident[:])

        # ---- projections ----
        TB = 512
        nblk = (EXT + TB - 1) // TB  # 9 (last block 256)
        for blk in range(nblk):
            t0 = blk * TB
            tb = min(TB, EXT - t0)
            for ec in range(4):
                ps = sim_ps.tile([P, TB], F32, tag="sim")
                for s in range(4):
                    nc.tensor.matmul(
                        ps[:, :tb],
                        lhsT=w_sb[:, s, D + ec * P : D + (ec + 1) * P],
                        rhs=x_sb[:, s, t0 : t0 + tb],
                        start=(s == 0),
                        stop=(s == 3),
                    )
                nc.vector.tensor_copy(k_sb[:, ec, t0 : t0 + tb], ps[:, :tb])

        # q feature-major, owned tokens only (shard tokens 128..4224)
        for blk in range(OWN // TB):  # 8
            t0 = blk * TB
            for ec in range(4):
                ps = sim_ps.tile([P, TB], F32, tag="sim")
                for s in range(4):
                    nc.tensor.matmul(
                        ps[:],
                        lhsT=w_sb[:, s, ec * P : (ec + 1) * P],
                        rhs=x_sb[:, s, HALO + t0 : HALO + t0 + TB],
                        start=(s == 0),
                        stop=(s == 3),
                    )
                nc.vector.tensor_copy(q_sb[:, ec, t0 : t0 + TB], ps[:])

        # v token-major per window
        for w in range(NWIN):
            ps = sim_ps.tile([P, TB], F32, tag="sim")
            for s in range(4):
                nc.tensor.matmul(
                    ps[:],
                    lhsT=x_sb[:, s, w * W : (w + 1) * W],
                    rhs=w_sb[:, s, 2 * D : 3 * D],
                    start=(s == 0),
                    stop=(s == 3),
                )
            nc.vector.tensor_copy(
                v_sb[:, w, :, 0:DH], ps.rearrange("p (h e) -> p h e", h=HEADS)
            )
            if 1 <= w <= NWIN - 2:
                # owned tokens are always in-sequence
                nc.vector.memset(v_sb[:, w, :, DH : DH + 1], 1.0)
            else:
                nc.vector.tensor_copy(
                    v_sb[:, w, :, DH : DH + 1],
                    vo_sb[:, w : w + 1, None].to_broadcast((P, HEADS, 1)),
                )

        # ---- attention ----
        # q_sb column t corresponds to shard window 1 + t//W.
        def qspan(kw):
            a = max(kw - 1, 1)
            b = min(kw + 1, NWIN - 2)
            return a, b

        e_tiles = {}
        for kw in range(NWIN):
            a, b = qspan(kw)
            span = (b - a + 1) * W
            qa = (a - 1) * W
            for c in range(4):
                sim = sim_ps.tile([P, 2, TB], F32, tag="sim")
                for hh in range(2):
                    off = hh * DH
                    nc.tensor.matmul(
                        sim[:, hh, :span],
                        lhsT=k_sb[off : off + DH, c, kw * W : (kw + 1) * W],
                        rhs=q_sb[off : off + DH, c, qa : qa + span],
                        start=True,
                        stop=True,
                    )
                e = epool.tile([P, 2, 3 * W], BF16, tag="e")
                nc.scalar.activation(
                    e[:, :, :span],
                    sim[:, :, :span],
                    mybir.ActivationFunctionType.Exp,
                    scale=SCALE,
                )
                e_tiles[(kw, c)] = e

            if kw < 2:
                continue
            # window w = kw-1 now has all its e tiles
            w = kw - 1
            att = att_ps.tile([P, HEADS, 2 * DH], F32, tag="att")
            for c in range(4):
                for hh in range(2):
                    h = 2 * c + hh
                    for kwi, kk in enumerate((w - 1, w, w + 1)):
                        e_t = e_tiles[(kk, c)]
                        rel = w - qspan(kk)[0]
                        nc.tensor.matmul(
                            att[:, h, 0 : DH + 1],
                            lhsT=e_t[:, hh, rel * W : (rel + 1) * W],
                            rhs=v_sb[:, kk, h, :],
                            start=(kwi == 0),
                            stop=(kwi == 2),
                        )
            recip = spool.tile([P, HEADS, 1], F32, tag="recip")
            nc.vector.reciprocal(recip[:], att[:, :, DH : DH + 1])
            attn = spool.tile([P, HEADS, DH], BF16, tag="attn")
            nc.vector.tensor_tensor(
                attn[:],
                att[:, :, 0:DH],
                recip[:, :, 0:1].to_broadcast((P, HEADS, DH)),
                mybir.AluOpType.mult,
            )
            attn_flat = attn.rearrange("p h d -> p (h d)")
            out_psum = out_ps.tile([P, D], F32, tag="outp")
            for c in range(4):
                tr = tr_ps.tile([P, W], BF16, tag="tr")
                nc.tensor.transpose(tr[:], attn_flat[:, c * W : (c + 1) * W], ident[:])
                fm = spool.tile([P, W], BF16, tag="fm")
                nc.vector.tensor_copy(fm[:], tr[:])
                nc.tensor.matmul(
                    out_psum[:],
                    lhsT=fm[:],
                    rhs=wo_sb[:, c, :],
                    start=(c == 0),
                    stop=(c == 3),
                )
            out_sb = opool.tile([P, D], F32, tag="osb")
            nc.vector.tensor_copy(out_sb[:], out_psum[:])
            wi = w - 1
            nc.sync.dma_start(out[wi * W : (wi + 1) * W, :], out_sb[:])


def _get_program():
    if "nc" not in _cached:
        _cached["nc"] = _build_program()
    return _cached["nc"]


def _make_in_maps(x, w_qkv, w_out):
    bf16 = ml_dtypes.bfloat16
    wqkvT = np.ascontiguousarray(np.asarray(w_qkv, np.float32).T).astype(bf16)
    woutT = np.ascontiguousarray(np.asarray(w_out, np.float32).T).astype(bf16)
    x = np.asarray(x, np.float32)
    in_maps = []
    for core in range(8):
        b, half = core // 2, core % 2
        s = half * OWN
        lo, hi = s - HALO, s + OWN + HALO
        xs = np.zeros((EXT, D), np.float32)
        src_lo, src_hi = max(lo, 0), min(hi, SEQ)
        xs[src_lo - lo : src_hi - lo] = x[b, src_lo:src_hi]
        xTc = np.ascontiguousarray(xs.T).astype(bf16)
        vo = np.zeros(EXT, np.float32)
        vo[src_lo - lo : src_hi - lo] = 1.0
        in_maps.append(
            {"xt": xTc, "wqkvt": wqkvT, "woutt": woutT, "vones": vo.astype(bf16)}
        )
    return in_maps


def run(x, w_qkv, w_out, trace=False, **spmd_kwargs):
    nc = _get_program()
    in_maps = _make_in_maps(x, w_qkv, w_out)
    res = run_bass_kernel_spmd(
        nc, in_maps, list(range(8)), trace=trace, **spmd_kwargs
    )
    out = np.empty((B, SEQ, D), np.float32)
    for core in range(8):
        b, half = core // 2, core % 2
        out[b, half * OWN : (half + 1) * OWN] = res.results[core]["out"]
    return out, res


def kernel(x, w_qkv, w_out):
    out, _ = run(x, w_qkv, w_out)
    return out


# revision 6
# speedup vs baseline: 2.7246x; 1.2173x over previous
"""Local windowed MHA (lucidrains LocalAttention, window=128, look_back=1,
look_fwd=1, non-causal) on 8 TRN2 NeuronCores.

Sharding: core = batch*2 + seq_half. Each core owns 4096 tokens of one
batch element plus a 128-token halo on each side (zero-padded at true
sequence edges). Attention is local, so shards are fully independent —
no collectives.

Per-core dataflow (zero DMA transposes; one PE transpose per head-pair):
  host passes xT (512, 4352) bf16, w_qkvT (512, 1536) bf16,
  w_outT (512, 512) bf16, vones (4352,) bf16 (1.0 for real tokens,
  0.0 for out-of-sequence pad tokens).
  - q_fm, k_fm: feature-major [feat, tok] projections (lhsT = w slice).
  - v65: token-major [128tok, head, 65] with column 64 = vones (pad
    indicator). Pad x is zero, so pad k and v are exactly zero.
  - QK, kw-batched: per (key-window kw, head): ONE matmul
    lhsT=k[kw] [64, 128], rhs=q over all query windows attending kw
    (N<=384) -> sim_T [128j, i] in psum; exp via ACT (scale=1/8, no max
    subtraction needed: |sim/8| < ~2).
  - A@V token-major: lhsT = e slice [128j, 128i], rhs = v65 [128j, 65]
    -> psum att[128i, head, 65] accumulated over 3 kws. Column 64 =
    sum_j e[j,i]*vones[j] = exact softmax denominator (pad keys excluded
    because their indicator is 0; pad v rows are exactly 0 so the
    numerator needs no correction). Reproduces the reference mask
    exactly.
  - normalize: one reciprocal [128, 8] + one multiply [128, 8, 64]
    per window -> attn token-major bf16.
  - PE-transpose per head-pair [128, 128] -> feature-major chunks;
    out-proj lhsT = chunk [128e, 128i], rhs = w_outT chunk [128e, 512m]
    -> psum [128i, 512m] token-major; contiguous DMA out.
"""

import sys

sys.path.insert(0, "/opt/trn_rl_repo")

import numpy as np
import ml_dtypes

import concourse.bass as bass
import concourse.tile as tile
import concourse.mybir as mybir
from concourse import bacc
from concourse.bass_utils import run_bass_kernel_spmd
from concourse.masks import make_identity

P = 128
HEADS = 8
DH = 64
W = 128  # window size
D = 512  # model dim
B = 4
SEQ = 8192
OWN = 4096  # tokens owned per core
HALO = 128
EXT = OWN + 2 * HALO  # 4352
NWIN = EXT // W  # 34 windows in shard (0 and 33 are halo)
OWIN = OWN // W  # 32 owned windows
F16 = mybir.dt.float16
F32 = mybir.dt.float32
SCALE = DH ** -0.5  # 0.125

_cached = {}


def _build_program():
    nc = bacc.Bacc("TRN2", target_bir_lowering=False, debug=False, num_devices=8)

    xT = nc.dram_tensor("xt", [D, EXT], F16, kind="ExternalInput").ap()
    wqkvT = nc.dram_tensor("wqkvt", [D, 3 * D], F16, kind="ExternalInput").ap()
    woutT = nc.dram_tensor("woutt", [D, D], F16, kind="ExternalInput").ap()
    vones = nc.dram_tensor("vones", [EXT], F16, kind="ExternalInput").ap()
    out = nc.dram_tensor("out", [OWN, D], F32, kind="ExternalOutput").ap()

    with tile.TileContext(nc) as tc:
        _emit(tc, xT, wqkvT, woutT, vones, out)

    nc.compile()
    return nc


def _emit(tc, xT, wqkvT, woutT, vones, out):
    nc = tc.nc
    import contextlib

    ctx = contextlib.ExitStack()
    with ctx:
        const = ctx.enter_context(tc.tile_pool(name="const", bufs=1))
        # PSUM budget (8 banks): sim pool 2 bufs x 2 banks = 4 (proj shares
        # the "sim" tag), att 1 x 2, tr 1 x <1, out 1 x 1.
        sim_ps = ctx.enter_context(tc.tile_pool(name="sim_ps", bufs=2, space="PSUM"))
        att_ps = ctx.enter_context(tc.tile_pool(name="att_ps", bufs=1, space="PSUM"))
        tr_ps = ctx.enter_context(tc.tile_pool(name="tr_ps", bufs=1, space="PSUM"))
        out_ps = ctx.enter_context(tc.tile_pool(name="out_ps", bufs=1, space="PSUM"))
        epool = ctx.enter_context(tc.tile_pool(name="epool", bufs=14))
        spool = ctx.enter_context(tc.tile_pool(name="spool", bufs=3))
        opool = ctx.enter_context(tc.tile_pool(name="opool", bufs=2))

        # ---- persistent SBUF tensors ----
        x_sb = const.tile([P, 4, EXT], F16)  # x feature-major: feat s*128+p
        w_sb = const.tile([P, 4, 3 * D], F16)  # w_qkvT: [d_in, e_out]
        wo_sb = const.tile([P, 4, D], F16)  # w_outT: [e_in, m_out]
        k_sb = const.tile([P, 4, EXT], F16)  # k feature-major
        q_sb = const.tile([P, 4, OWN], F16)  # q feature-major (owned only)
        v_sb = const.tile([P, NWIN, HEADS, DH + 1], F16)  # v tok-major + den col
        vo_sb = const.tile([P, NWIN], F16)  # pad indicator per (tok%128, win)
        ident = const.tile([P, P], F16)

        xT_r = xT.rearrange("(s p) t -> p s t", p=P)
        nc.sync.dma_start(w_sb[:], wqkvT.rearrange("(s p) e -> p s e", p=P))
        nc.sync.dma_start(wo_sb[:], woutT.rearrange("(s p) m -> p s m", p=P))
        nc.sync.dma_start(vo_sb[:], vones.rearrange("(w p) -> p w", p=P))
        make_identity(nc, ident[:])

        # ---- projections, pipelined per 512-token x block ----
        TB = 512
        nblk = (EXT + TB - 1) // TB  # 9 (last block 256)

        def q_block(bq):
            t0 = bq * TB
            for ec in range(4):
                ps = sim_ps.tile([P, TB], F32, tag="sim")
                for s in range(4):
                    nc.tensor.matmul(
                        ps[:],
                        lhsT=w_sb[:, s, ec * P : (ec + 1) * P],
                        rhs=x_sb[:, s, HALO + t0 : HALO + t0 + TB],
                        start=(s == 0),
                        stop=(s == 3),
                    )
                nc.vector.tensor_copy(q_sb[:, ec, t0 : t0 + TB], ps[:])

        for blk in range(nblk):
            t0 = blk * TB
            tb = min(TB, EXT - t0)
            nc.sync.dma_start(x_sb[:, :, t0 : t0 + tb], xT_r[:, :, t0 : t0 + tb])
            # k feature-major chunks for this block
            for ec in range(4):
                ps = sim_ps.tile([P, TB], F32, tag="sim")
                for s in range(4):
                    nc.tensor.matmul(
                        ps[:, :tb],
                        lhsT=w_sb[:, s, D + ec * P : D + (ec + 1) * P],
                        rhs=x_sb[:, s, t0 : t0 + tb],
                        start=(s == 0),
                        stop=(s == 3),
                    )
                nc.vector.tensor_copy(k_sb[:, ec, t0 : t0 + tb], ps[:, :tb])
            # v token-major windows in this block
            for w in range(t0 // W, (t0 + tb) // W):
                ps = sim_ps.tile([P, TB], F32, tag="sim")
                for s in range(4):
                    nc.tensor.matmul(
                        ps[:],
                        lhsT=x_sb[:, s, w * W : (w + 1) * W],
                        rhs=w_sb[:, s, 2 * D : 3 * D],
                        start=(s == 0),
                        stop=(s == 3),
                    )
                nc.vector.tensor_copy(
                    v_sb[:, w, :, 0:DH], ps.rearrange("p (h e) -> p h e", h=HEADS)
                )
                if 1 <= w <= NWIN - 2:
                    # owned tokens are always in-sequence
                    nc.vector.memset(v_sb[:, w, :, DH : DH + 1], 1.0)
                else:
                    nc.vector.tensor_copy(
                        v_sb[:, w, :, DH : DH + 1],
                        vo_sb[:, w : w + 1, None].to_broadcast((P, HEADS, 1)),
                    )
            # q block bq spans x cols [HALO+bq*TB, HALO+(bq+1)*TB) i.e.
            # x blocks bq and bq+1 — emit once both are loaded
            if blk >= 1:
                q_block(blk - 1)

        # ---- attention ----
        # q_sb column t corresponds to shard window 1 + t//W.
        def qspan(kw):
            a = max(kw - 1, 1)
            b = min(kw + 1, NWIN - 2)
            return a, b

        e_tiles = {}
        for kw in range(NWIN):
            a, b = qspan(kw)
            span = (b - a + 1) * W
            qa = (a - 1) * W
            for c in range(4):
                sim = sim_ps.tile([P, 2, TB], F32, tag="sim")
                for hh in range(2):
                    off = hh * DH
                    nc.tensor.matmul(
                        sim[:, hh, :span],
                        lhsT=k_sb[off : off + DH, c, kw * W : (kw + 1) * W],
                        rhs=q_sb[off : off + DH, c, qa : qa + span],
                        start=True,
                        stop=True,
                    )
                e = epool.tile([P, 2, 3 * W], F16, tag="e")
                nc.scalar.activation(
                    e[:, :, :span],
                    sim[:, :, :span],
                    mybir.ActivationFunctionType.Exp,
                    scale=SCALE,
                )
                e_tiles[(kw, c)] = e

            if kw < 2:
                continue
            # window w = kw-1 now has all its e tiles
            w = kw - 1
            att = att_ps.tile([P, HEADS, 2 * DH], F32, tag="att")
            for c in range(4):
                for hh in range(2):
                    h = 2 * c + hh
                    for kwi, kk in enumerate((w - 1, w, w + 1)):
                        e_t = e_tiles[(kk, c)]
                        rel = w - qspan(kk)[0]
                        nc.tensor.matmul(
                            att[:, h, 0 : DH + 1],
                            lhsT=e_t[:, hh, rel * W : (rel + 1) * W],
                            rhs=v_sb[:, kk, h, :],
                            start=(kwi == 0),
                            stop=(kwi == 2),
                        )
            recip = spool.tile([P, HEADS, 1], F32, tag="recip")
            nc.vector.reciprocal(recip[:], att[:, :, DH : DH + 1])
            attn = spool.tile([P, HEADS, DH], F16, tag="attn")
            nc.vector.tensor_tensor(
                attn[:],
                att[:, :, 0:DH],
                recip[:, :, 0:1].to_broadcast((P, HEADS, DH)),
                mybir.AluOpType.mult,
            )
            attn_flat = attn.rearrange("p h d -> p (h d)")
            out_psum = out_ps.tile([P, D], F32, tag="outp")
            for c in range(4):
                tr = tr_ps.tile([P, W], F16, tag="tr")
                nc.tensor.transpose(tr[:], attn_flat[:, c * W : (c + 1) * W], ident[:])
                fm = spool.tile([P, W], F16, tag="fm")
                nc.vector.tensor_copy(fm[:], tr[:])
                nc.tensor.matmul(
                    out_psum[:],
                    lhsT=fm[:],
                    rhs=wo_sb[:, c, :],
                    start=(c == 0),
                    stop=(c == 3),
                )
            out_sb = opool.tile([P, D], F32, tag="osb")
            nc.vector.tensor_copy(out_sb[:], out_psum[:])
            wi = w - 1
            nc.sync.dma_start(out[wi * W : (wi + 1) * W, :], out_sb[:])


def _get_program():
    if "nc" not in _cached:
        _cached["nc"] = _build_program()
    return _cached["nc"]


def _make_in_maps(x, w_qkv, w_out):
    f16 = np.float16
    wqkvT = np.ascontiguousarray(np.asarray(w_qkv, np.float32).T).astype(f16)
    woutT = np.ascontiguousarray(np.asarray(w_out, np.float32).T).astype(f16)
    x = np.asarray(x, np.float32)
    in_maps = []
    for core in range(8):
        b, half = core // 2, core % 2
        s = half * OWN
        lo, hi = s - HALO, s + OWN + HALO
        xs = np.zeros((EXT, D), np.float32)
        src_lo, src_hi = max(lo, 0), min(hi, SEQ)
        xs[src_lo - lo : src_hi - lo] = x[b, src_lo:src_hi]
        xTc = np.ascontiguousarray(xs.T).astype(f16)
        vo = np.zeros(EXT, np.float32)
        vo[src_lo - lo : src_hi - lo] = 1.0
        in_maps.append(
            {"xt": xTc, "wqkvt": wqkvT, "woutt": woutT, "vones": vo.astype(f16)}
        )
    return in_maps


def run(x, w_qkv, w_out, trace=False, **spmd_kwargs):
    nc = _get_program()
    in_maps = _make_in_maps(x, w_qkv, w_out)
    res = run_bass_kernel_spmd(
        nc, in_maps, list(range(8)), trace=trace, **spmd_kwargs
    )
    out = np.empty((B, SEQ, D), np.float32)
    for core in range(8):
        b, half = core // 2, core % 2
        out[b, half * OWN : (half + 1) * OWN] = res.results[core]["out"]
    return out, res


def kernel(x, w_qkv, w_out):
    out, _ = run(x, w_qkv, w_out)
    return out
